# revision 7
# baseline (speedup 1.0000x reference)
"""Trainium2 Bass kernel for nn_AttentionLayer (dense_transformer).

reference:
    scores  = einsum('bqd,bkd->bqk', padded_seqs, encoder_padded_seqs) / sqrt(D)
    A       = softmax(scores, axis=2)                    -> output[1]
    context = einsum('bqk,bkd->bqd', A, encoder_padded_seqs)
    out     = tanh(concat([padded_seqs, context], -1) @ W.T + b) * mask  -> output[0]

Sharding: data-parallel over batch, B=32 -> 4 batches per NeuronCore on 8
cores; W/b replicated.  No collectives.

Device strategy (per batch, S=D=512, P=128):
  - Host pre-transposes Q->Q^T and sends both K and K^T, all in bf16 (fp32
    matmul is 4x slower on the PE and fp32 has no DMA-transpose path; fp32
    accumulation in PSUM keeps rel-err ~5e-3, gate is 2e-2).
  - scores(q,k):  matmul(lhsT=Q^T chunks, rhs=K^T)            16 MM
  - softmax along free axis: Exp on ScalarE (scale=1/sqrt(D); no max
    subtraction needed: scores ~ N(0,1)), batched row-sum + reciprocal +
    broadcast-AP normalize on VectorE.
  - A^T via TensorE transpose (16 x 128x128), context^T(d,q) =
    matmul(lhsT=K chunks, rhs=A^T)                            16 MM
  - linear out(q,j) = matmul(lhsT=[Q^T;context^T] chunks, rhs=W^T) 32 MM
  - epilogue: +bias (free-axis broadcast tile, VectorE), Tanh (ScalarE),
    *mask (broadcast-AP multiply, VectorE).
  - Software pipeline: scores/softmax of batch b+1 are emitted before the
    consume stages (A^T/context/linear) of batch b, so the PE never stalls
    on the softmax latency.
  - Outputs are written bf16 and upcast to f32 on the host (halves output
    DMA traffic; quantization ~0.2% rel).
"""

import json
import math
import sys

import numpy as np

sys.path.insert(0, "/opt/trn_rl_repo")

import ml_dtypes  # noqa: E402

import concourse.bass as bass  # noqa: E402
import concourse.mybir as mybir  # noqa: E402
import concourse.tile as tile  # noqa: E402
from concourse.bass_utils import run_bass_kernel_spmd  # noqa: E402
from concourse.masks import make_identity  # noqa: E402

# ---------------------------------------------------------------------------
# Workaround: this container's walrus (CoreV3GenImpl::setupSyncWait) rejects
# any instruction carrying more than one semaphore wait, while Tile freely
# emits multi-wait instructions.  Legalize at BIR-JSON level: hoist extra
# waits onto injected same-engine NoOps immediately before the instruction
# (engine streams execute in order, so semantics are preserved).
# ---------------------------------------------------------------------------


def _legalize_bir_waits(bir: bytes) -> bytes:
    m = json.loads(bir)
    ctr = 0
    changed = False
    for func in m.get("functions", []):
        for blk in func.get("blocks", []):
            new_instrs = []
            for ins in blk.get("instructions", []):
                si = ins.get("sync_info")
                waits = (si or {}).get("on_wait") or []
                if len(waits) > 1:
                    changed = True
                    for w in waits[:-1]:
                        ctr += 1
                        new_instrs.append(
                            {
                                "debug": ins.get("debug", 0),
                                "engine": ins["engine"],
                                "ins": [],
                                "outs": [],
                                "name": f"WSPLIT-{ctr}",
                                "opcode": "NoOp",
                                "sync_info": {"on_update": [], "on_wait": [w]},
                                "text_hint": "wait_split",
                            }
                        )
                    si["on_wait"] = [waits[-1]]
                new_instrs.append(ins)
            blk["instructions"] = new_instrs
    if not changed:
        return bir
    return json.dumps(m).encode()


def _install_waitfix():
    if getattr(bass.Bass, "_waitfix_installed", False):
        return
    orig = bass.Bass.to_json_bytes

    def to_json_bytes(self):
        return _legalize_bir_waits(orig(self))

    bass.Bass.to_json_bytes = to_json_bytes
    bass.Bass._waitfix_installed = True


_install_waitfix()

# ---------------------------------------------------------------------------
# Problem constants (hardcoded per spec: B=32, SD=SE=D=512, 8 cores)
# ---------------------------------------------------------------------------
B, S, D = 32, 512, 512
N_CORES = 8
BL = B // N_CORES  # batches per core
P = 128
NQ = S // P  # q chunks
ND = D // P  # d chunks
NC2 = (2 * D) // P  # concat-feature chunks
SCALE = 1.0 / math.sqrt(D)

BF16 = mybir.dt.bfloat16
F32 = mybir.dt.float32


def _build_nc():
    nc = bass.Bass()

    qt = nc.declare_dram_parameter("qt", [BL, D, S], BF16, isOutput=False)
    k = nc.declare_dram_parameter("k", [BL, S, D], BF16, isOutput=False)
    kt = nc.declare_dram_parameter("kt", [BL, D, S], BF16, isOutput=False)
    wt = nc.declare_dram_parameter("wt", [2 * D, D], BF16, isOutput=False)
    bias = nc.declare_dram_parameter("bias", [D], F32, isOutput=False)
    mask = nc.declare_dram_parameter("mask", [BL, S], F32, isOutput=False)
    out_m = nc.declare_dram_parameter("out_m", [BL, S, D], BF16, isOutput=True)
    out_w = nc.declare_dram_parameter("out_w", [BL, S, S], BF16, isOutput=True)

    qt_r = [qt[ib].rearrange("(o p) q -> o p q", p=P) for ib in range(BL)]
    kt_r = [kt[ib].rearrange("(o p) s -> o p s", p=P) for ib in range(BL)]
    k_r = [k[ib].rearrange("(o p) d -> o p d", p=P) for ib in range(BL)]

    with tile.TileContext(nc) as tc:
        with (
            tc.tile_pool(name="singles", bufs=1) as singles,
            tc.tile_pool(name="ins", bufs=2) as ins_pool,
            tc.tile_pool(name="mids", bufs=2) as mids,
            tc.tile_pool(name="psum_s", bufs=4, space="PSUM") as psum_s,
            tc.tile_pool(name="psum_cl", bufs=2, space="PSUM") as psum_cl,
        ):
            state = {}
            setup = {}

            def emit_setup():
                # trigger the Exp/Tanh ACT table load early, during input DMA
                dummy = singles.tile([P, 1], F32)
                nc.vector.memset(dummy, 0.0)
                nc.scalar.activation(
                    out=dummy, in_=dummy, func=mybir.ActivationFunctionType.Exp
                )

                wt_sb = singles.tile([P, NC2, D], BF16)
                for co in range(NC2):
                    nc.gpsimd.dma_start(
                        out=wt_sb[:, co, :],
                        in_=wt.rearrange("(o p) j -> o p j", p=P)[co],
                    )

                bias_bc = singles.tile([P, D], F32)
                _b = bias[:]
                bias_ap = bass.AP(
                    tensor=_b.tensor, offset=_b.offset, ap=[[0, P], *_b.ap]
                )
                nc.gpsimd.dma_start(out=bias_bc, in_=bias_ap)

                mask_sb = singles.tile([P, BL, NQ], F32)
                nc.gpsimd.dma_start(
                    out=mask_sb, in_=mask.rearrange("b (qo p) -> p b qo", p=P)
                )
                setup.update(wt_sb=wt_sb, bias_bc=bias_bc, mask_sb=mask_sb)

            for ib in range(BL + 1):
                # ---------------- stage A: load + scores + softmax ----------
                if ib < BL:
                    qts = []
                    kts = []
                    ks = []
                    for o in range(ND):
                        tq = ins_pool.tile([P, S], BF16, tag=f"qt{o}")
                        nc.sync.dma_start(out=tq, in_=qt_r[ib][o])
                        qts.append(tq)
                        tk = ins_pool.tile([P, S], BF16, tag=f"kt{o}")
                        nc.sync.dma_start(out=tk, in_=kt_r[ib][o])
                        kts.append(tk)
                    for o in range(NQ):
                        t = ins_pool.tile([P, S], BF16, tag=f"k{o}")
                        nc.sync.dma_start(out=t, in_=k_r[ib][o])
                        ks.append(t)
                    if ib == 0:
                        emit_setup()

                    e_sb = mids.tile([P, NQ, S], BF16, tag="e")
                    a_bf = mids.tile([P, NQ, S], BF16, tag="a")
                    sums = mids.tile([P, NQ], F32, tag="sums")
                    recip = mids.tile([P, NQ], F32, tag="recip")
                    for qi in range(NQ):
                        ps_sc = psum_s.tile([P, S], F32, tag="ps")
                        for do in range(ND):
                            nc.tensor.matmul(
                                ps_sc,
                                lhsT=qts[do][:, qi * P : (qi + 1) * P],
                                rhs=kts[do],
                                start=(do == 0),
                                stop=(do == ND - 1),
                            )
                        nc.scalar.activation(
                            out=e_sb[:, qi, :],
                            in_=ps_sc,
                            func=mybir.ActivationFunctionType.Exp,
                            scale=SCALE,
                        )
                    nc.vector.reduce_sum(
                        out=sums, in_=e_sb, axis=mybir.AxisListType.X
                    )
                    nc.vector.reciprocal(out=recip, in_=sums)
                    nc.vector.tensor_tensor(
                        a_bf,
                        e_sb,
                        recip[:, :, None].to_broadcast((P, NQ, S)),
                        mybir.AluOpType.mult,
                    )
                    nc.gpsimd.dma_start(
                        out=out_w[ib].rearrange("(o p) s -> p o s", p=P),
                        in_=a_bf,
                    )
                    state[ib] = (qts, ks, a_bf)

                # ------------- stage B: A^T, context, linear, epilogue ------
                if ib >= 1:
                    jb = ib - 1
                    qts, ks, a_bf = state.pop(jb)
                    wt_sb = setup["wt_sb"]

                    at_bf = mids.tile([P, NQ, S], BF16, tag="at")
                    for ko in range(NQ):
                        for qi in range(NQ):
                            nc.scalar.dma_start(
                                out=at_bf[:, ko, qi * P : (qi + 1) * P],
                                in_=a_bf[:, qi, ko * P : (ko + 1) * P],
                                transpose=True,
                            )

                    ct_bf = mids.tile([P, ND, S], BF16, tag="ct")
                    for do in range(ND):
                        ps_c = psum_cl.tile([P, S], F32, tag="pcl")
                        for ko in range(NQ):
                            nc.tensor.matmul(
                                ps_c,
                                lhsT=ks[ko][:, do * P : (do + 1) * P],
                                rhs=at_bf[:, ko, :],
                                start=(ko == 0),
                                stop=(ko == NQ - 1),
                            )
                        nc.scalar.activation(
                            out=ct_bf[:, do, :],
                            in_=ps_c,
                            func=mybir.ActivationFunctionType.Copy,
                        )

                    o_bf = mids.tile([P, NQ, D], BF16, tag="om")
                    for qi in range(NQ):
                        ps_o = psum_cl.tile([P, S], F32, tag="pcl")
                        for co in range(NC2):
                            src = (
                                qts[co][:, qi * P : (qi + 1) * P]
                                if co < ND
                                else ct_bf[:, co - ND, qi * P : (qi + 1) * P]
                            )
                            nc.tensor.matmul(
                                ps_o,
                                lhsT=src,
                                rhs=wt_sb[:, co, :],
                                start=(co == 0),
                                stop=(co == NC2 - 1),
                            )
                        tmp = mids.tile([P, D], F32, tag="tmp")
                        nc.vector.tensor_add(
                            out=tmp, in0=ps_o, in1=setup["bias_bc"]
                        )
                        nc.scalar.activation(
                            out=o_bf[:, qi, :],
                            in_=tmp,
                            func=mybir.ActivationFunctionType.Tanh,
                        )
                        nc.vector.tensor_scalar_mul(
                            out=o_bf[:, qi, :],
                            in0=o_bf[:, qi, :],
                            scalar1=setup["mask_sb"][:, jb, qi : qi + 1],
                        )
                        nc.gpsimd.dma_start(
                            out=out_m[jb].rearrange("(o p) d -> p o d", p=P)[
                                :, qi, :
                            ],
                            in_=o_bf[:, qi, :],
                        )

    return nc


_CACHE: dict = {}


def _get_nc():
    if "nc" not in _CACHE:
        _CACHE["nc"] = _build_nc()
    return _CACHE["nc"]


def _run(in_maps, trace=False):
    nc = _get_nc()
    return run_bass_kernel_spmd(
        nc, in_maps, core_ids=list(range(N_CORES)), trace=trace
    )


def make_in_maps(padded_seqs, encoder_padded_seqs, decoder_mask, W, b):
    bf = ml_dtypes.bfloat16
    qt_full = np.ascontiguousarray(
        np.transpose(padded_seqs, (0, 2, 1))
    ).astype(bf)
    k_full = np.ascontiguousarray(encoder_padded_seqs).astype(bf)
    kt_full = np.ascontiguousarray(
        np.transpose(encoder_padded_seqs, (0, 2, 1))
    ).astype(bf)
    wt_np = np.ascontiguousarray(W.T).astype(bf)
    b_np = np.ascontiguousarray(b).astype(np.float32)
    mask_np = np.ascontiguousarray(decoder_mask[..., 0]).astype(np.float32)

    in_maps = []
    for c in range(N_CORES):
        sl = slice(c * BL, (c + 1) * BL)
        in_maps.append(
            {
                "qt": qt_full[sl],
                "k": k_full[sl],
                "kt": kt_full[sl],
                "wt": wt_np,
                "bias": b_np,
                "mask": mask_np[sl],
            }
        )
    return in_maps


def kernel(padded_seqs, encoder_padded_seqs, decoder_mask, W, b):
    in_maps = make_in_maps(padded_seqs, encoder_padded_seqs, decoder_mask, W, b)
    res = _run(in_maps)
    out_m = np.concatenate(
        [np.asarray(res.results[c]["out_m"]) for c in range(N_CORES)], axis=0
    ).astype(np.float32)
    out_w = np.concatenate(
        [np.asarray(res.results[c]["out_w"]) for c in range(N_CORES)], axis=0
    ).astype(np.float32)
    return out_m, out_w


# revision 11
# speedup vs baseline: 2.1474x; 2.1474x over previous
"""Trainium2 Bass kernel for nn_AttentionLayer (dense_transformer).

reference:
    scores  = einsum('bqd,bkd->bqk', padded_seqs, encoder_padded_seqs) / sqrt(D)
    A       = softmax(scores, axis=2)                    -> output[1]
    context = einsum('bqk,bkd->bqd', A, encoder_padded_seqs)
    out     = tanh(concat([padded_seqs, context], -1) @ W.T + b) * mask  -> output[0]

Sharding: data-parallel over batch, B=32 -> 4 batches per NeuronCore on 8
cores; W/b replicated.  No collectives.

Device strategy (per batch, S=D=512, P=128):
  - Host pre-transposes Q->Q^T and sends both K and K^T, all in bf16 (fp32
    matmul is 4x slower on the PE and fp32 has no DMA-transpose path; fp32
    accumulation in PSUM keeps rel-err ~5e-3, gate is 2e-2).
  - scores(q,k):  matmul(lhsT=Q^T chunks, rhs=K^T)            16 MM
  - softmax along free axis: Exp on ScalarE (scale=1/sqrt(D); no max
    subtraction needed: scores ~ N(0,1)), batched row-sum + reciprocal +
    broadcast-AP normalize on VectorE.
  - A^T via TensorE transpose (16 x 128x128), context^T(d,q) =
    matmul(lhsT=K chunks, rhs=A^T)                            16 MM
  - linear out(q,j) = matmul(lhsT=[Q^T;context^T] chunks, rhs=W^T) 32 MM
  - epilogue: +bias (free-axis broadcast tile, VectorE), Tanh (ScalarE),
    *mask (broadcast-AP multiply, VectorE).
  - Software pipeline: scores/softmax of batch b+1 are emitted before the
    consume stages (A^T/context/linear) of batch b, so the PE never stalls
    on the softmax latency.
  - Outputs are written bf16 and upcast to f32 on the host (halves output
    DMA traffic; quantization ~0.2% rel).
"""

import json
import math
import sys

import numpy as np

sys.path.insert(0, "/opt/trn_rl_repo")

import ml_dtypes  # noqa: E402

import concourse.bass as bass  # noqa: E402
import concourse.mybir as mybir  # noqa: E402
import concourse.tile as tile  # noqa: E402
from concourse.bass_utils import run_bass_kernel_spmd  # noqa: E402
from concourse.masks import make_identity  # noqa: E402

# ---------------------------------------------------------------------------
# Workaround: this container's walrus (CoreV3GenImpl::setupSyncWait) rejects
# any instruction carrying more than one semaphore wait, while Tile freely
# emits multi-wait instructions.  Legalize at BIR-JSON level: hoist extra
# waits onto injected same-engine NoOps immediately before the instruction
# (engine streams execute in order, so semantics are preserved).
# ---------------------------------------------------------------------------


def _legalize_bir_waits(bir: bytes) -> bytes:
    m = json.loads(bir)
    ctr = 0
    changed = False
    for func in m.get("functions", []):
        for blk in func.get("blocks", []):
            new_instrs = []
            for ins in blk.get("instructions", []):
                si = ins.get("sync_info")
                waits = (si or {}).get("on_wait") or []
                if len(waits) > 1:
                    changed = True
                    for w in waits[:-1]:
                        ctr += 1
                        new_instrs.append(
                            {
                                "debug": ins.get("debug", 0),
                                "engine": ins["engine"],
                                "ins": [],
                                "outs": [],
                                "name": f"WSPLIT-{ctr}",
                                "opcode": "NoOp",
                                "sync_info": {"on_update": [], "on_wait": [w]},
                                "text_hint": "wait_split",
                            }
                        )
                    si["on_wait"] = [waits[-1]]
                new_instrs.append(ins)
            blk["instructions"] = new_instrs
    if not changed:
        return bir
    return json.dumps(m).encode()


def _install_waitfix():
    if getattr(bass.Bass, "_waitfix_installed", False):
        return
    orig = bass.Bass.to_json_bytes

    def to_json_bytes(self):
        return _legalize_bir_waits(orig(self))

    bass.Bass.to_json_bytes = to_json_bytes
    bass.Bass._waitfix_installed = True


_install_waitfix()

# ---------------------------------------------------------------------------
# Problem constants (hardcoded per spec: B=32, SD=SE=D=512, 8 cores)
# ---------------------------------------------------------------------------
B, S, D = 32, 512, 512
N_CORES = 8
BL = B // N_CORES  # batches per core
P = 128
NQ = S // P  # q chunks
ND = D // P  # d chunks
NC2 = (2 * D) // P  # concat-feature chunks
SCALE = 1.0 / math.sqrt(D)

BF16 = mybir.dt.bfloat16
F32 = mybir.dt.float32


def _build_nc():
    nc = bass.Bass()

    qt = nc.declare_dram_parameter("qt", [BL, D, S], BF16, isOutput=False)
    k = nc.declare_dram_parameter("k", [BL, S, D], BF16, isOutput=False)
    kt = nc.declare_dram_parameter("kt", [BL, D, S], BF16, isOutput=False)
    wt = nc.declare_dram_parameter("wt", [2 * D, D], BF16, isOutput=False)
    bias = nc.declare_dram_parameter("bias", [D], F32, isOutput=False)
    mask = nc.declare_dram_parameter("mask", [BL, S], F32, isOutput=False)
    out_m = nc.declare_dram_parameter("out_m", [BL, S, D], BF16, isOutput=True)
    out_w = nc.declare_dram_parameter("out_w", [BL, S, S], BF16, isOutput=True)

    qt_r = [qt[ib].rearrange("(o p) q -> o p q", p=P) for ib in range(BL)]
    kt_r = [kt[ib].rearrange("(o p) s -> o p s", p=P) for ib in range(BL)]
    k_r = [k[ib].rearrange("(o p) d -> o p d", p=P) for ib in range(BL)]

    with tile.TileContext(nc) as tc:
        with (
            tc.tile_pool(name="singles", bufs=1) as singles,
            tc.tile_pool(name="ins", bufs=2) as ins_pool,
            tc.tile_pool(name="mids", bufs=2) as mids,
            tc.tile_pool(name="psum_s", bufs=4, space="PSUM") as psum_s,
            tc.tile_pool(name="psum_t", bufs=2, space="PSUM") as psum_t,
            tc.tile_pool(name="psum_cl", bufs=2, space="PSUM") as psum_cl,
        ):
            state = {}
            setup = {}

            def emit_setup():
                ident = singles.tile([P, P], BF16)
                make_identity(nc, ident)

                wt_sb = singles.tile([P, NC2, D], BF16)
                for co in range(NC2):
                    nc.scalar.dma_start(
                        out=wt_sb[:, co, :],
                        in_=wt.rearrange("(o p) j -> o p j", p=P)[co],
                    )

                bias_bc = singles.tile([P, D], F32)
                _b = bias[:]
                bias_ap = bass.AP(
                    tensor=_b.tensor, offset=_b.offset, ap=[[0, P], *_b.ap]
                )
                nc.gpsimd.dma_start(out=bias_bc, in_=bias_ap)

                mask_sb = singles.tile([P, BL, NQ], F32)
                nc.gpsimd.dma_start(
                    out=mask_sb, in_=mask.rearrange("b (qo p) -> p b qo", p=P)
                )
                setup.update(
                    ident=ident, wt_sb=wt_sb, bias_bc=bias_bc, mask_sb=mask_sb
                )

            for ib in range(BL + 1):
                # ---------------- stage A: load + scores + softmax ----------
                if ib < BL:
                    qts = []
                    kts = []
                    ks = []
                    for o in range(ND):
                        t = ins_pool.tile([P, S], BF16, tag=f"qt{o}")
                        nc.sync.dma_start(out=t, in_=qt_r[ib][o])
                        qts.append(t)
                    for o in range(ND):
                        t = ins_pool.tile([P, S], BF16, tag=f"kt{o}")
                        nc.scalar.dma_start(out=t, in_=kt_r[ib][o])
                        kts.append(t)
                    for o in range(NQ):
                        t = ins_pool.tile([P, S], BF16, tag=f"k{o}")
                        nc.sync.dma_start(out=t, in_=k_r[ib][o])
                        ks.append(t)
                    if ib == 0:
                        emit_setup()

                    e_sb = mids.tile([P, NQ, S], BF16, tag="e")
                    a_bf = mids.tile([P, NQ, S], BF16, tag="a")
                    sums = mids.tile([P, NQ], F32, tag="sums")
                    recip = mids.tile([P, NQ], F32, tag="recip")
                    for qi in range(NQ):
                        ps_sc = psum_s.tile([P, S], F32, tag="ps")
                        for do in range(ND):
                            nc.tensor.matmul(
                                ps_sc,
                                lhsT=qts[do][:, qi * P : (qi + 1) * P],
                                rhs=kts[do],
                                start=(do == 0),
                                stop=(do == ND - 1),
                            )
                        nc.scalar.activation(
                            out=e_sb[:, qi, :],
                            in_=ps_sc,
                            func=mybir.ActivationFunctionType.Exp,
                            scale=SCALE,
                        )
                    nc.vector.reduce_sum(
                        out=sums, in_=e_sb, axis=mybir.AxisListType.X
                    )
                    nc.vector.reciprocal(out=recip, in_=sums)
                    nc.vector.tensor_tensor(
                        a_bf,
                        e_sb,
                        recip[:, :, None].to_broadcast((P, NQ, S)),
                        mybir.AluOpType.mult,
                    )
                    nc.sync.dma_start(
                        out=out_w[ib].rearrange("(o p) s -> p o s", p=P),
                        in_=a_bf,
                    )
                    state[ib] = (qts, ks, a_bf)

                # ------------- stage B: A^T, context, linear, epilogue ------
                if ib >= 1:
                    jb = ib - 1
                    qts, ks, a_bf = state.pop(jb)
                    ident = setup["ident"]
                    wt_sb = setup["wt_sb"]

                    at_bf = mids.tile([P, NQ, S], BF16, tag="at")
                    for ko in range(NQ):
                        ps_tr = psum_t.tile([P, S], BF16, tag="pst")
                        for qi in range(NQ):
                            nc.tensor.transpose(
                                out=ps_tr[:, qi * P : (qi + 1) * P],
                                in_=a_bf[:, qi, ko * P : (ko + 1) * P],
                                identity=ident,
                            )
                        nc.vector.tensor_copy(out=at_bf[:, ko, :], in_=ps_tr)

                    ct_bf = mids.tile([P, ND, S], BF16, tag="ct")
                    for do in range(ND):
                        ps_c = psum_cl.tile([P, S], F32, tag="pcl")
                        for ko in range(NQ):
                            nc.tensor.matmul(
                                ps_c,
                                lhsT=ks[ko][:, do * P : (do + 1) * P],
                                rhs=at_bf[:, ko, :],
                                start=(ko == 0),
                                stop=(ko == NQ - 1),
                            )
                        nc.scalar.activation(
                            out=ct_bf[:, do, :],
                            in_=ps_c,
                            func=mybir.ActivationFunctionType.Copy,
                        )

                    o_bf = mids.tile([P, NQ, D], BF16, tag="om")
                    for qi in range(NQ):
                        ps_o = psum_cl.tile([P, S], F32, tag="pcl")
                        for co in range(NC2):
                            src = (
                                qts[co][:, qi * P : (qi + 1) * P]
                                if co < ND
                                else ct_bf[:, co - ND, qi * P : (qi + 1) * P]
                            )
                            nc.tensor.matmul(
                                ps_o,
                                lhsT=src,
                                rhs=wt_sb[:, co, :],
                                start=(co == 0),
                                stop=(co == NC2 - 1),
                            )
                        tmp = mids.tile([P, D], F32, tag="tmp")
                        nc.vector.tensor_add(
                            out=tmp, in0=ps_o, in1=setup["bias_bc"]
                        )
                        nc.scalar.activation(
                            out=o_bf[:, qi, :],
                            in_=tmp,
                            func=mybir.ActivationFunctionType.Tanh,
                        )
                    nc.vector.tensor_tensor(
                        o_bf,
                        o_bf,
                        setup["mask_sb"][:, jb, :, None].to_broadcast(
                            (P, NQ, D)
                        ),
                        mybir.AluOpType.mult,
                    )
                    nc.scalar.dma_start(
                        out=out_m[jb].rearrange("(o p) d -> p o d", p=P),
                        in_=o_bf,
                    )

    return nc


_CACHE: dict = {}


def _get_nc():
    if "nc" not in _CACHE:
        _CACHE["nc"] = _build_nc()
    return _CACHE["nc"]


def _run(in_maps, trace=False):
    nc = _get_nc()
    return run_bass_kernel_spmd(
        nc, in_maps, core_ids=list(range(N_CORES)), trace=trace
    )


def make_in_maps(padded_seqs, encoder_padded_seqs, decoder_mask, W, b):
    bf = ml_dtypes.bfloat16
    qt_full = np.ascontiguousarray(
        np.transpose(padded_seqs, (0, 2, 1))
    ).astype(bf)
    k_full = np.ascontiguousarray(encoder_padded_seqs).astype(bf)
    kt_full = np.ascontiguousarray(
        np.transpose(encoder_padded_seqs, (0, 2, 1))
    ).astype(bf)
    wt_np = np.ascontiguousarray(W.T).astype(bf)
    b_np = np.ascontiguousarray(b).astype(np.float32)
    mask_np = np.ascontiguousarray(decoder_mask[..., 0]).astype(np.float32)

    in_maps = []
    for c in range(N_CORES):
        sl = slice(c * BL, (c + 1) * BL)
        in_maps.append(
            {
                "qt": qt_full[sl],
                "k": k_full[sl],
                "kt": kt_full[sl],
                "wt": wt_np,
                "bias": b_np,
                "mask": mask_np[sl],
            }
        )
    return in_maps


def kernel(padded_seqs, encoder_padded_seqs, decoder_mask, W, b):
    in_maps = make_in_maps(padded_seqs, encoder_padded_seqs, decoder_mask, W, b)
    res = _run(in_maps)
    out_m = np.concatenate(
        [np.asarray(res.results[c]["out_m"]) for c in range(N_CORES)], axis=0
    ).astype(np.float32)
    out_w = np.concatenate(
        [np.asarray(res.results[c]["out_w"]) for c in range(N_CORES)], axis=0
    ).astype(np.float32)
    return out_m, out_w
